# revision 16
# baseline (speedup 1.0000x reference)
"""Sharded causal self-attention block (GPT-2 style) for 8 Trainium2 NeuronCores.

kernel(x, w_attn, b_attn, w_proj, b_proj) -> (out, present)
  x       [4, 2048, 1024] f32
  w_attn  [1024, 3072] f32 (fan_in_fan_out: y = x @ W + b), b_attn [3072]
  w_proj  [1024, 1024] f32, b_proj [1024]
  out     [4, 2048, 1024] f32
  present [2, 4, 16, 2048, 64] f32  (k then v, [B,H,S,dh])

Sharding: core c -> batch c//2, head-group c%2 (8 of 16 heads each).  Each core
computes qkv for its heads, causal attention, and a partial output projection
over its 512 channels; the host sums the two per-batch partials and adds b_proj.

Per-core device program (all matmuls in float32r: full PE rate, ~1e-4 rel err):
  - x^T via PE transposes; qT/kT computed head-dim-on-partitions, v token-major
    with an appended ones column per head.
  - scores computed transposed (s[j,i] = k_j.q_i) per 128x512 causal block, two
    heads paired in one 2-bank PSUM tile; exp(s/8) fused from PSUM on the
    scalar engine (one instruction per pair); causal boundary masked with one
    gpsimd affine_select per head.
  - A.V matmul uses lhsT=[v | 1] (M=65): PSUM row 64 accumulates the softmax
    denominator for free; reciprocal is partition-broadcast and fused into the
    PSUM eviction (division deferred past the exp/AV matmuls).
  - attention runs i-block-outer so the output projection for an i-block's
    tokens (contraction over all heads) interleaves into the same phase.
  - AV is software-pipelined (LAG groups) so the PE never waits on the exp.
"""
import sys
if '/opt/trn_rl_repo' not in sys.path:
    sys.path.insert(0, '/opt/trn_rl_repo')

import numpy as np
from contextlib import ExitStack

import concourse.mybir as mybir
import concourse.tile as tile
from concourse import bacc
from concourse.masks import make_identity

F32 = mybir.dt.float32
AF = mybir.ActivationFunctionType
ALU = mybir.AluOpType

N_HEAD = 16
B_, S_, D_ = 4, 2048, 1024
N_CORES = 8


def build_core_program(S=2048, D=1024, H_loc=8, dh=64, n_cores=8,
                       mm_dt=mybir.dt.float32r, LAG=2, CREDIT=0.3,
                       PAV_BUFS=2, PPJ_BUFS=2, EXPP_BUFS=5):
    P = 128
    IB = 512                      # i-block (query block) size
    assert S % IB == 0 and D % P == 0 and dh == 64 and H_loc % 2 == 0
    HP = H_loc // 2               # head pairs
    CQK = H_loc * dh              # q (=k) columns per core
    CV = H_loc * dh
    CH = H_loc * dh               # proj input channels per core
    KO = D // P
    NT = S // P
    NTB = S // IB
    NIB = S // IB
    NCT = CQK // P                # col-tiles for each of q and k (== HP)
    CHT = CH // P
    NJT = IB // P
    NOH = max(1, D // IB)
    OCW = min(D, IB)

    FR = mm_dt
    nc = bacc.Bacc("TRN2", target_bir_lowering=False, debug=False,
                   num_devices=n_cores)
    x = nc.dram_tensor("x", [S, D], FR, kind="ExternalInput").ap()
    wqk = nc.dram_tensor("wqk", [D, 2 * CQK], FR, kind="ExternalInput").ap()
    wv = nc.dram_tensor("wv", [D, CV], FR, kind="ExternalInput").ap()
    wp = nc.dram_tensor("wp", [CH, D], FR, kind="ExternalInput").ap()
    bqk = nc.dram_tensor("bqk", [1, 2 * CQK], F32, kind="ExternalInput").ap()
    bv = nc.dram_tensor("bv", [1, CV], F32, kind="ExternalInput").ap()
    out_p = nc.dram_tensor("out_p", [S, D], F32, kind="ExternalOutput").ap()
    pk = nc.dram_tensor("pk", [H_loc, S, dh], F32, kind="ExternalOutput").ap()
    pv = nc.dram_tensor("pv", [H_loc, S, dh], F32, kind="ExternalOutput").ap()

    # head-major views of the present outputs: [p, ttile, head, dh]
    pk_t = pk.rearrange("h (t p) d -> p t h d", p=P)
    pv_t = pv.rearrange("h (t p) d -> p t h d", p=P)
    wqk_t = wqk.rearrange("(ko p) c -> p ko c", p=P)

    scale = float(1.0 / np.sqrt(dh))

    with tile.TileContext(nc) as tc, ExitStack() as ctx:
        st0x = ctx.enter_context(tc.tile_pool(name="st0x", bufs=3))
        # first x chunk DMA goes out before anything else
        x_chunks = {}
        xc = st0x.tile([P, D], FR, tag="x_in", name="xc_first")
        nc.sync.dma_start(xc, x[0:P].rearrange("(o p) d -> p (o d)", p=P))
        x_chunks[0] = xc

        const = ctx.enter_context(tc.tile_pool(name="const", bufs=1))
        identity32 = const.tile([P, P], F32)
        make_identity(nc, identity32)
        identity = const.tile([P, P], FR)
        nc.vector.tensor_copy(identity, identity32)
        bqk_sb = const.tile([P, 2 * NCT], F32)
        nc.sync.dma_start(bqk_sb, bqk.rearrange("o (ct p) -> p (o ct)", p=P))
        bv_row = const.tile([1, CV], F32)
        nc.sync.dma_start(bv_row, bv)
        bv_rep = const.tile([P, CV], F32)
        nc.gpsimd.partition_broadcast(bv_rep, bv_row)

        big = ctx.enter_context(tc.tile_pool(name="big", bufs=1))
        qTp = tc.tile_pool(name="qTp", bufs=1)
        qT = qTp.__enter__().tile([P, NCT, S], FR)
        kT = big.tile([P, NCT, S], FR)
        vext = big.tile([P, NT, H_loc, dh + 1], FR)    # token-major v | ones col
        nc.vector.tensor_scalar(vext[:, :, :, dh:dh + 1],
                                bv_rep[:, 0:NT * H_loc], 0.0, 1.0,
                                ALU.mult, ALU.add)

        # ---------------- stage 0: x^T, qkv (two 512-blocks per wqk load) ---
        TP = 2 * IB               # token-pair block (1024)
        NTP = S // TP
        with tc.tile_pool(name="st0t", bufs=1) as st0t, \
             tc.tile_pool(name="wvp", bufs=2) as wvp, \
             tc.tile_pool(name="wqp", bufs=3) as wqp, \
             tc.tile_pool(name="ps0", bufs=3, space="PSUM") as ps0, \
             tc.tile_pool(name="pstr", bufs=2, space="PSUM") as pstr:
            for T in range(NTP):
                xT = st0t.tile([P, KO, TP], FR, tag="xT")
                wts = {}

                def wt_prefetch(ct):
                    wt = wqp.tile([P, KO, P], FR, tag="wqk", name=f"wt{ct}")
                    nc.sync.dma_start(wt, wqk_t[:, :, ct * P:(ct + 1) * P])
                    wts[ct] = wt

                wt_prefetch(0)
                for tcn in range(TP // P):
                    gt = T * (TP // P) + tcn
                    if gt in x_chunks:
                        xc = x_chunks.pop(gt)
                    else:
                        xc = st0x.tile([P, D], FR, tag="x_in", name=f"xc{gt}")
                        nc.sync.dma_start(
                            xc, x[gt * P:(gt + 1) * P].rearrange(
                                "(o p) d -> p (o d)", p=P))
                    if tcn == TP // P - 2:
                        wt_prefetch(1)
                    for dt_ in range(KO):
                        pst = pstr.tile([P, P], FR, tag="tr")
                        nc.tensor.transpose(pst, xc[:, dt_ * P:(dt_ + 1) * P],
                                            identity)
                        nc.scalar.copy(xT[:, dt_, tcn * P:(tcn + 1) * P], pst)
                # q and k column tiles (transposed orientation)
                for ct in range(2 * NCT):
                    if ct + 2 < 2 * NCT:
                        wt_prefetch(ct + 2)
                    wt = wts.pop(ct)
                    for half in range(TP // IB):
                        ps = ps0.tile([P, IB], F32, tag="mm")
                        for k in range(KO):
                            nc.tensor.matmul(
                                ps, wt[:, k],
                                xT[:, k, half * IB:(half + 1) * IB],
                                start=(k == 0), stop=(k == KO - 1))
                        dest = (qT if ct < NCT else kT)[
                            :, ct % NCT, T * TP + half * IB:T * TP + (half + 1) * IB]
                        nc.vector.tensor_scalar_add(dest, ps, bqk_sb[:, ct:ct + 1])
                # v (token-major, wv streamed in halves) + present-v out
                for vh in range(2):
                    wvt = wvp.tile([P, KO, CV // 2], FR, tag="wv")
                    nc.sync.dma_start(
                        wvt, wv.rearrange("(ko p) c -> p ko c", p=P)
                        [:, :, vh * (CV // 2):(vh + 1) * (CV // 2)])
                    for tcn in range(TP // P):
                        ps = ps0.tile([P, CV // 2], F32, tag="mmv")
                        for k in range(KO):
                            nc.tensor.matmul(ps, xT[:, k, tcn * P:(tcn + 1) * P],
                                             wvt[:, k],
                                             start=(k == 0), stop=(k == KO - 1))
                        gt = T * (TP // P) + tcn
                        h0 = vh * (H_loc // 2)
                        nc.vector.tensor_tensor(
                            vext[:, gt, h0:h0 + H_loc // 2, 0:dh], ps,
                            bv_rep[:, vh * (CV // 2):(vh + 1) * (CV // 2)], ALU.add)

        # ---------------- stage A: attention + proj (i-block outer) --------
        aT = big.tile([P, CHT, S], FR)
        with tc.tile_pool(name="expp", bufs=EXPP_BUFS) as expp, \
             tc.tile_pool(name="dpool", bufs=2) as dpool, \
             tc.tile_pool(name="rrpool", bufs=2) as rrpool, \
             tc.tile_pool(name="kout", bufs=3) as kout, \
             tc.tile_pool(name="wpp", bufs=1) as wpp, \
             tc.tile_pool(name="outp", bufs=3) as outpool, \
             tc.tile_pool(name="psc", bufs=2, space="PSUM") as psc, \
             tc.tile_pool(name="pav", bufs=PAV_BUFS, space="PSUM") as pav, \
             tc.tile_pool(name="ppj", bufs=PPJ_BUFS, space="PSUM") as ppj:
            wp_sb = wpp.tile([P, CHT, D], FR)
            nc.sync.dma_start(wp_sb, wp.rearrange("(ko p) c -> p ko c", p=P))

            def pk_unit(hp, jt):
                pst = ppj.tile([P, P], FR, tag="fill", name="pkt")
                nc.tensor.transpose(pst, kT[:, hp, jt * P:(jt + 1) * P],
                                    identity)
                ko = kout.tile([P, P], F32, tag="ko")
                nc.any.tensor_copy(ko, pst[:, 0:P])
                nc.sync.dma_start(pk_t[:, jt, 2 * hp:2 * hp + 2, :], ko)

            def proj_unit(tt, oh):
                ps = ppj.tile([P, OCW], F32, tag="fill", name="pjt")
                for k in range(CHT):
                    nc.tensor.matmul(ps[:, 0:OCW], aT[:, k, tt * P:(tt + 1) * P],
                                     wp_sb[:, k, oh * OCW:(oh + 1) * OCW],
                                     start=(k == 0), stop=(k == CHT - 1))
                ot = outpool.tile([P, OCW], F32, tag="ot")
                nc.any.tensor_copy(ot, ps[:, 0:OCW])
                nc.sync.dma_start(
                    out_p[tt * P:(tt + 1) * P, oh * OCW:(oh + 1) * OCW], ot)

            # filler queue: proj of completed i-blocks + present-k transposes
            # + present-v DMAs, sprinkled between attention groups to absorb
            # PE wait slots
            fillers = []
            fill_state = {"credit": 0.0}
            for hp_ in range(HP):
                for jt_ in range(NT):
                    fillers.append(lambda hp2=hp_, jt=jt_: pk_unit(hp2, jt))
                    if jt_ % 4 == 0:
                        gt = hp_ * 4 + jt_ // 4
                        fillers.append(lambda gt=gt: nc.sync.dma_start(
                            pv_t[:, gt], vext[:, gt, :, 0:dh].bitcast(F32)))

            def emit_fillers(credit):
                fill_state["credit"] += credit
                while fillers and fill_state["credit"] >= 1.0:
                    fillers.pop(0)()
                    fill_state["credit"] -= 1.0

            for ib in range(NIB):
                njt = NJT * ib + NJT
                for hp in range(HP):
                    av = [pav.tile([P, IB], F32, tag="av", name=f"av{_X}")
                          for _X in range(2)]

                    def emit_av(j, ex, cs, njt=njt, hp=hp, av=av):
                        for X in range(2):
                            nc.tensor.matmul(
                                av[X][0:dh + 1, cs],
                                vext[:, j, 2 * hp + X, :],
                                ex[:, X, cs],
                                start=(j == 0), stop=(j == njt - 1))

                    pending = []
                    for j in range(njt):
                        delta = j * P - ib * IB
                        # pad restricted blocks to >=256 cols (fp32r rate rule)
                        lo = min(max(0, delta), IB - 256)
                        cs = slice(lo, IB)
                        ecs = slice(max(0, delta), IB)
                        ssc = psc.tile([P, 2, IB], F32, tag="sc")
                        for X in range(2):
                            b0 = X * 64
                            nc.tensor.matmul(
                                ssc[:, X, cs],
                                kT[b0:b0 + 64, hp, j * P:(j + 1) * P],
                                qT[b0:b0 + 64, hp, ib * IB:(ib + 1) * IB][:, cs],
                                start=True, stop=True)
                        ex = expp.tile([P, 2, IB], FR, tag="exp")
                        nc.scalar.activation(ex[:, :, ecs], ssc[:, :, ecs], AF.Exp,
                                             scale=scale)
                        if delta >= 0:
                            # zero cols [lo, delta) and the triangular part of
                            # [delta, delta+P): keep iff (c - delta - p) >= 0
                            for X in range(2):
                                nc.gpsimd.affine_select(
                                    out=ex[:, X, lo:delta + P],
                                    in_=ex[:, X, lo:delta + P],
                                    compare_op=ALU.is_ge, fill=0.0,
                                    base=lo - delta, channel_multiplier=-1,
                                    pattern=[[1, delta + P - lo]])
                        pending.append((j, ex, cs))
                        if len(pending) > LAG:
                            emit_av(*pending.pop(0))
                        emit_fillers(CREDIT)
                    for p_ in pending:
                        emit_av(*p_)
                    for X in range(2):
                        avs = dpool.tile([dh, IB], F32, tag="avs")
                        nc.vector.tensor_copy(avs, av[X][0:dh, :])
                        dX = dpool.tile([1, IB], F32, tag="d")
                        nc.vector.tensor_copy(dX, av[X][dh:dh + 1, :])
                        nc.vector.reciprocal(dX, dX)
                        rr = rrpool.tile([64, IB], F32, tag="rr")
                        nc.gpsimd.partition_broadcast(rr, dX)
                        nc.vector.tensor_tensor(
                            aT[X * 64:X * 64 + 64, hp, ib * IB:(ib + 1) * IB],
                            avs[0:dh, :], rr, ALU.mult)
                    # a couple of present-k transposes between head pairs
                    emit_fillers(1.0)
                # queue fillers: this i-block's proj + one head-pair present-k
                for tt in range(ib * NJT, (ib + 1) * NJT):
                    for oh in range(NOH):
                        fillers.append(lambda tt=tt, oh=oh: proj_unit(tt, oh))
            for f in fillers:
                f()
        qTp.__exit__(None, None, None)

    nc.compile()
    return nc


def shard_inputs(x, w_attn, b_attn, n_cores=N_CORES, n_head=N_HEAD):
    """Hybrid shard: core c -> batch c//2, head-group c%2."""
    B, S, D = x.shape
    dh = D // n_head
    groups = n_cores // B
    Hg = n_head // groups
    C = Hg * dh
    in_maps = []
    for c in range(n_cores):
        b, g = divmod(c, groups)
        base = g * C
        wqk_c = np.ascontiguousarray(
            np.concatenate([w_attn[:, base:base + C],
                            w_attn[:, D + base:D + base + C]], axis=1))
        wv_c = np.ascontiguousarray(w_attn[:, 2 * D + base:2 * D + base + C])
        bqk_c = np.ascontiguousarray(
            np.concatenate([b_attn[base:base + C],
                            b_attn[D + base:D + base + C]])[None, :])
        bv_c = np.ascontiguousarray(b_attn[2 * D + base:2 * D + base + C][None, :])
        in_maps.append({"x": np.ascontiguousarray(x[b]), "wqk": wqk_c,
                        "wv": wv_c, "bqk": bqk_c, "bv": bv_c})
    return in_maps


def shard_wproj(w_proj, in_maps, n_cores=N_CORES, n_head=N_HEAD, B=B_):
    groups = n_cores // B
    C = (n_head // groups) * (w_proj.shape[1] // n_head)
    for c in range(n_cores):
        g = c % groups
        in_maps[c]["wp"] = np.ascontiguousarray(w_proj[g * C:(g + 1) * C, :])
    return in_maps


def gather_outputs(results, b_proj, B, S, D, n_cores=N_CORES, n_head=N_HEAD):
    groups = n_cores // B
    Hg = n_head // groups
    dh = D // n_head
    out = np.zeros((B, S, D), dtype=np.float32)
    present = np.zeros((2, B, n_head, S, dh), dtype=np.float32)
    for c in range(n_cores):
        b, g = divmod(c, groups)
        out[b] += results[c]["out_p"]
        present[0, b, g * Hg:(g + 1) * Hg] = results[c]["pk"]
        present[1, b, g * Hg:(g + 1) * Hg] = results[c]["pv"]
    out += np.asarray(b_proj, np.float32)[None, None, :]
    return out, present


_NC_CACHE = {}


def _get_nc():
    if "nc" not in _NC_CACHE:
        _NC_CACHE["nc"] = build_core_program(
            S=S_, D=D_, H_loc=N_HEAD * B_ // N_CORES, dh=D_ // N_HEAD,
            n_cores=N_CORES)
    return _NC_CACHE["nc"]


def kernel(x, w_attn, b_attn, w_proj, b_proj):
    from concourse import bass_utils
    x = np.asarray(x, np.float32)
    w_attn = np.asarray(w_attn, np.float32)
    b_attn = np.asarray(b_attn, np.float32)
    w_proj = np.asarray(w_proj, np.float32)
    b_proj = np.asarray(b_proj, np.float32)
    B, S, D = x.shape

    nc = _get_nc()
    in_maps = shard_inputs(x, w_attn, b_attn)
    in_maps = shard_wproj(w_proj, in_maps)
    res = bass_utils.run_bass_kernel_spmd(nc, in_maps, core_ids=list(range(N_CORES)))
    return gather_outputs(res.results, b_proj, B, S, D)


# revision 25
# speedup vs baseline: 1.0389x; 1.0389x over previous
"""Sharded causal self-attention block (GPT-2 style) for 8 Trainium2 NeuronCores.

kernel(x, w_attn, b_attn, w_proj, b_proj) -> (out, present)
  x       [4, 2048, 1024] f32
  w_attn  [1024, 3072] f32 (fan_in_fan_out: y = x @ W + b), b_attn [3072]
  w_proj  [1024, 1024] f32, b_proj [1024]
  out     [4, 2048, 1024] f32
  present [2, 4, 16, 2048, 64] f32  (k then v, [B,H,S,dh])

Sharding: core c -> batch c//2, head-group c%2 (8 of 16 heads each).  Each core
computes qkv for its heads, causal attention, and a partial output projection
over its 512 channels; the host sums the two per-batch partials and adds b_proj.

Per-core device program (all matmuls in float32r: full PE rate, ~1e-4 rel err):
  - x^T via PE transposes; qT/kT computed head-dim-on-partitions, v token-major
    with an appended ones column per head.
  - scores computed transposed (s[j,i] = k_j.q_i) per 128x512 causal block, two
    heads paired in one 2-bank PSUM tile; exp(s/8) fused from PSUM on the
    scalar engine (one instruction per pair); causal boundary masked with one
    gpsimd affine_select per head.
  - A.V matmul uses lhsT=[v | 1] (M=65): PSUM row 64 accumulates the softmax
    denominator for free; reciprocal is partition-broadcast and fused into the
    PSUM eviction (division deferred past the exp/AV matmuls).
  - attention runs i-block-outer so the output projection for an i-block's
    tokens (contraction over all heads) interleaves into the same phase.
  - AV is software-pipelined (LAG groups) so the PE never waits on the exp.
"""
import sys
if '/opt/trn_rl_repo' not in sys.path:
    sys.path.insert(0, '/opt/trn_rl_repo')

import numpy as np
from contextlib import ExitStack

import concourse.mybir as mybir
import concourse.tile as tile
from concourse import bacc
from concourse.masks import make_identity

F32 = mybir.dt.float32
AF = mybir.ActivationFunctionType
ALU = mybir.AluOpType

N_HEAD = 16
B_, S_, D_ = 4, 2048, 1024
N_CORES = 8


def build_core_program(S=2048, D=1024, H_loc=8, dh=64, n_cores=8,
                       mm_dt=mybir.dt.float32r, LAG=2, CREDIT=0.5,
                       PAV_BUFS=2, PPJ_BUFS=2, EXPP_BUFS=5, IB_DESC=False):
    P = 128
    IB = 512                      # i-block (query block) size
    assert S % IB == 0 and D % P == 0 and dh == 64 and H_loc % 2 == 0
    HP = H_loc // 2               # head pairs
    CQK = H_loc * dh              # q (=k) columns per core
    CV = H_loc * dh
    CH = H_loc * dh               # proj input channels per core
    KO = D // P
    NT = S // P
    NTB = S // IB
    NIB = S // IB
    NCT = CQK // P                # col-tiles for each of q and k (== HP)
    CHT = CH // P
    NJT = IB // P
    NOH = max(1, D // IB)
    OCW = min(D, IB)

    FR = mm_dt
    nc = bacc.Bacc("TRN2", target_bir_lowering=False, debug=False,
                   num_devices=n_cores)
    x = nc.dram_tensor("x", [S, D], FR, kind="ExternalInput").ap()
    wqk = nc.dram_tensor("wqk", [D, 2 * CQK], FR, kind="ExternalInput").ap()
    wv = nc.dram_tensor("wv", [D, CV], FR, kind="ExternalInput").ap()
    wp = nc.dram_tensor("wp", [CH, D], FR, kind="ExternalInput").ap()
    bqk = nc.dram_tensor("bqk", [1, 2 * CQK], F32, kind="ExternalInput").ap()
    bv = nc.dram_tensor("bv", [1, CV], F32, kind="ExternalInput").ap()
    out_p = nc.dram_tensor("out_p", [S, D], F32, kind="ExternalOutput").ap()
    pk = nc.dram_tensor("pk", [H_loc, S, dh], F32, kind="ExternalOutput").ap()
    pv = nc.dram_tensor("pv", [H_loc, S, dh], F32, kind="ExternalOutput").ap()

    # head-major views of the present outputs: [p, ttile, head, dh]
    pk_t = pk.rearrange("h (t p) d -> p t h d", p=P)
    pv_t = pv.rearrange("h (t p) d -> p t h d", p=P)
    wqk_t = wqk.rearrange("(ko p) c -> p ko c", p=P)

    scale = float(1.0 / np.sqrt(dh))

    with tile.TileContext(nc) as tc, ExitStack() as ctx:
        st0x = ctx.enter_context(tc.tile_pool(name="st0x", bufs=3))
        # first x chunk DMAs go out before anything else
        x_chunks = {}

        def xc_prefetch(gt, NT_=S // P):
            if gt >= NT_ or gt in x_chunks:
                return
            t_ = st0x.tile([P, D], FR, tag="x_in", name=f"xc{gt}")
            nc.sync.dma_start(
                t_, x[gt * P:(gt + 1) * P].rearrange("(o p) d -> p (o d)", p=P))
            x_chunks[gt] = t_

        for _g in range(3):
            xc_prefetch(_g)

        const = ctx.enter_context(tc.tile_pool(name="const", bufs=1))
        identity32 = const.tile([P, P], F32)
        make_identity(nc, identity32)
        identity = const.tile([P, P], FR)
        nc.vector.tensor_copy(identity, identity32)
        bqk_sb = const.tile([P, 2 * NCT], F32)
        nc.sync.dma_start(bqk_sb, bqk.rearrange("o (ct p) -> p (o ct)", p=P))
        bv_row = const.tile([1, CV], F32)
        nc.sync.dma_start(bv_row, bv)
        bv_rep = const.tile([P, CV], F32)
        nc.gpsimd.partition_broadcast(bv_rep, bv_row)

        big = ctx.enter_context(tc.tile_pool(name="big", bufs=1))
        qTp = tc.tile_pool(name="qTp", bufs=1)
        qT = qTp.__enter__().tile([P, NCT, S], FR)
        kT = big.tile([P, NCT, S], FR)
        vext = big.tile([P, NT, H_loc, dh + 1], FR)    # token-major v | ones col
        nc.vector.tensor_scalar(vext[:, :, :, dh:dh + 1],
                                bv_rep[:, 0:NT * H_loc], 0.0, 1.0,
                                ALU.mult, ALU.add)

        # ---------------- stage 0: x^T, qkv (two 512-blocks per wqk load) ---
        TP = 2 * IB               # token-pair block (1024)
        NTP = S // TP
        with tc.tile_pool(name="st0t", bufs=1) as st0t, \
             tc.tile_pool(name="wvp", bufs=2) as wvp, \
             tc.tile_pool(name="wqp", bufs=3) as wqp, \
             tc.tile_pool(name="ps0", bufs=3, space="PSUM") as ps0, \
             tc.tile_pool(name="pstr", bufs=3, space="PSUM") as pstr:
            for T in range(NTP):
                xT = st0t.tile([P, KO, TP], FR, tag="xT")
                wts = {}

                def wt_prefetch(ct):
                    wt = wqp.tile([P, KO, P], FR, tag="wqk", name=f"wt{ct}")
                    nc.sync.dma_start(wt, wqk_t[:, :, ct * P:(ct + 1) * P])
                    wts[ct] = wt

                wt_prefetch(0)
                for tcn in range(TP // P):
                    gt = T * (TP // P) + tcn
                    xc_prefetch(gt)
                    xc = x_chunks.pop(gt)
                    if tcn == TP // P - 2:
                        wt_prefetch(1)
                    else:
                        xc_prefetch(gt + 2)
                    for dt_ in range(KO):
                        pst = pstr.tile([P, P], FR, tag="tr")
                        nc.tensor.transpose(pst, xc[:, dt_ * P:(dt_ + 1) * P],
                                            identity)
                        if dt_ % 2 == 0:
                            nc.scalar.copy(xT[:, dt_, tcn * P:(tcn + 1) * P], pst)
                        else:
                            nc.vector.tensor_copy(
                                xT[:, dt_, tcn * P:(tcn + 1) * P], pst)
                # q and k column tiles (transposed orientation)
                for ct in range(2 * NCT):
                    if ct + 2 < 2 * NCT:
                        wt_prefetch(ct + 2)
                    # pull next T-pair's x chunks in during the matmul phase
                    xc_prefetch((T + 1) * (TP // P) + ct)
                    wt = wts.pop(ct)
                    for half in range(TP // IB):
                        ps = ps0.tile([P, IB], F32, tag="mm")
                        for k in range(KO):
                            nc.tensor.matmul(
                                ps, wt[:, k],
                                xT[:, k, half * IB:(half + 1) * IB],
                                start=(k == 0), stop=(k == KO - 1))
                        dest = (qT if ct < NCT else kT)[
                            :, ct % NCT, T * TP + half * IB:T * TP + (half + 1) * IB]
                        nc.vector.tensor_scalar_add(dest, ps, bqk_sb[:, ct:ct + 1])
                # v (token-major, wv streamed in halves) + present-v out
                for vh in range(2):
                    wvt = wvp.tile([P, KO, CV // 2], FR, tag="wv")
                    nc.sync.dma_start(
                        wvt, wv.rearrange("(ko p) c -> p ko c", p=P)
                        [:, :, vh * (CV // 2):(vh + 1) * (CV // 2)])
                    for tcn in range(TP // P):
                        ps = ps0.tile([P, CV // 2], F32, tag="mmv", bufs=2)
                        for k in range(KO):
                            nc.tensor.matmul(ps, xT[:, k, tcn * P:(tcn + 1) * P],
                                             wvt[:, k],
                                             start=(k == 0), stop=(k == KO - 1))
                        gt = T * (TP // P) + tcn
                        h0 = vh * (H_loc // 2)
                        nc.vector.tensor_tensor(
                            vext[:, gt, h0:h0 + H_loc // 2, 0:dh], ps,
                            bv_rep[:, vh * (CV // 2):(vh + 1) * (CV // 2)], ALU.add)

        # ---------------- stage A: attention + proj (i-block outer) --------
        aT = big.tile([P, CHT, S], FR)
        with tc.tile_pool(name="expp", bufs=EXPP_BUFS) as expp, \
             tc.tile_pool(name="dpool", bufs=2) as dpool, \
             tc.tile_pool(name="rrpool", bufs=2) as rrpool, \
             tc.tile_pool(name="kout", bufs=3) as kout, \
             tc.tile_pool(name="wpp", bufs=1) as wpp, \
             tc.tile_pool(name="outp", bufs=3) as outpool, \
             tc.tile_pool(name="psc", bufs=2, space="PSUM") as psc, \
             tc.tile_pool(name="pav", bufs=PAV_BUFS, space="PSUM") as pav, \
             tc.tile_pool(name="ppj", bufs=PPJ_BUFS, space="PSUM") as ppj:
            wp_sb = wpp.tile([P, CHT, D], FR)
            nc.sync.dma_start(wp_sb, wp.rearrange("(ko p) c -> p ko c", p=P))

            def pk_unit(hp, jt):
                pst = ppj.tile([P, P], FR, tag="fill", name="pkt")
                nc.tensor.transpose(pst, kT[:, hp, jt * P:(jt + 1) * P],
                                    identity)
                ko = kout.tile([P, P], F32, tag="ko")
                nc.any.tensor_copy(ko, pst[:, 0:P])
                nc.sync.dma_start(pk_t[:, jt, 2 * hp:2 * hp + 2, :], ko)

            def proj_unit(tt, oh):
                ps = ppj.tile([P, OCW], F32, tag="fill", name="pjt")
                for k in range(CHT):
                    nc.tensor.matmul(ps[:, 0:OCW], aT[:, k, tt * P:(tt + 1) * P],
                                     wp_sb[:, k, oh * OCW:(oh + 1) * OCW],
                                     start=(k == 0), stop=(k == CHT - 1))
                ot = outpool.tile([P, OCW], F32, tag="ot")
                nc.any.tensor_copy(ot, ps[:, 0:OCW])
                nc.sync.dma_start(
                    out_p[tt * P:(tt + 1) * P, oh * OCW:(oh + 1) * OCW], ot)

            # filler queue: proj of completed i-blocks + present-k transposes
            # + present-v DMAs, sprinkled between attention groups to absorb
            # PE wait slots
            fillers = []
            fill_state = {"credit": 0.0}
            for hp_ in range(HP):
                for jt_ in range(NT):
                    fillers.append(lambda hp2=hp_, jt=jt_: pk_unit(hp2, jt))
                    if jt_ % 4 == 0:
                        gt = hp_ * 4 + jt_ // 4
                        fillers.append(lambda gt=gt: nc.sync.dma_start(
                            pv_t[:, gt], vext[:, gt, :, 0:dh].bitcast(F32)))

            def emit_fillers(credit):
                fill_state["credit"] += credit
                while fillers and fill_state["credit"] >= 1.0:
                    fillers.pop(0)()
                    fill_state["credit"] -= 1.0

            for ib in (range(NIB - 1, -1, -1) if IB_DESC else range(NIB)):
                njt = NJT * ib + NJT
                for hp in range(HP):
                    av = [pav.tile([P, IB], F32, tag="av", name=f"av{_X}")
                          for _X in range(2)]

                    def emit_av(j, ex, cs, njt=njt, hp=hp, av=av):
                        for X in range(2):
                            nc.tensor.matmul(
                                av[X][0:dh + 1, cs],
                                vext[:, j, 2 * hp + X, :],
                                ex[:, X, cs],
                                start=(j == 0), stop=(j == njt - 1))

                    pending = []
                    for j in range(njt):
                        delta = j * P - ib * IB
                        # pad restricted blocks to >=256 cols (fp32r rate rule)
                        lo = min(max(0, delta), IB - 256)
                        cs = slice(lo, IB)
                        ecs = slice(max(0, delta), IB)
                        ssc = psc.tile([P, 2, IB], F32, tag="sc")
                        for X in range(2):
                            b0 = X * 64
                            nc.tensor.matmul(
                                ssc[:, X, cs],
                                kT[b0:b0 + 64, hp, j * P:(j + 1) * P],
                                qT[b0:b0 + 64, hp, ib * IB:(ib + 1) * IB][:, cs],
                                start=True, stop=True)
                        ex = expp.tile([P, 2, IB], FR, tag="exp")
                        nc.scalar.activation(ex[:, :, ecs], ssc[:, :, ecs], AF.Exp,
                                             scale=scale)
                        if delta >= 0:
                            # zero cols [lo, delta) and the triangular part of
                            # [delta, delta+P): keep iff (c - delta - p) >= 0
                            for X in range(2):
                                nc.gpsimd.affine_select(
                                    out=ex[:, X, lo:delta + P],
                                    in_=ex[:, X, lo:delta + P],
                                    compare_op=ALU.is_ge, fill=0.0,
                                    base=lo - delta, channel_multiplier=-1,
                                    pattern=[[1, delta + P - lo]])
                        pending.append((j, ex, cs))
                        if len(pending) > LAG:
                            emit_av(*pending.pop(0))
                        emit_fillers(CREDIT)
                    for p_ in pending:
                        emit_av(*p_)
                    for X in range(2):
                        avs = dpool.tile([dh, IB], F32, tag="avs")
                        nc.vector.tensor_copy(avs, av[X][0:dh, :])
                        dX = dpool.tile([1, IB], F32, tag="d")
                        nc.vector.tensor_copy(dX, av[X][dh:dh + 1, :])
                        nc.vector.reciprocal(dX, dX)
                        rr = rrpool.tile([64, IB], F32, tag="rr")
                        nc.gpsimd.partition_broadcast(rr, dX)
                        nc.vector.tensor_tensor(
                            aT[X * 64:X * 64 + 64, hp, ib * IB:(ib + 1) * IB],
                            avs[0:dh, :], rr, ALU.mult)
                    # a couple of present-k transposes between head pairs
                    emit_fillers(1.0)
                # queue fillers: this i-block's proj + one head-pair present-k
                for tt in range(ib * NJT, (ib + 1) * NJT):
                    for oh in range(NOH):
                        fillers.append(lambda tt=tt, oh=oh: proj_unit(tt, oh))
            for f in fillers:
                f()
        qTp.__exit__(None, None, None)

    nc.compile()
    return nc


def shard_inputs(x, w_attn, b_attn, n_cores=N_CORES, n_head=N_HEAD):
    """Hybrid shard: core c -> batch c//2, head-group c%2."""
    B, S, D = x.shape
    dh = D // n_head
    groups = n_cores // B
    Hg = n_head // groups
    C = Hg * dh
    in_maps = []
    for c in range(n_cores):
        b, g = divmod(c, groups)
        base = g * C
        wqk_c = np.ascontiguousarray(
            np.concatenate([w_attn[:, base:base + C],
                            w_attn[:, D + base:D + base + C]], axis=1))
        wv_c = np.ascontiguousarray(w_attn[:, 2 * D + base:2 * D + base + C])
        bqk_c = np.ascontiguousarray(
            np.concatenate([b_attn[base:base + C],
                            b_attn[D + base:D + base + C]])[None, :])
        bv_c = np.ascontiguousarray(b_attn[2 * D + base:2 * D + base + C][None, :])
        in_maps.append({"x": np.ascontiguousarray(x[b]), "wqk": wqk_c,
                        "wv": wv_c, "bqk": bqk_c, "bv": bv_c})
    return in_maps


def shard_wproj(w_proj, in_maps, n_cores=N_CORES, n_head=N_HEAD, B=B_):
    groups = n_cores // B
    C = (n_head // groups) * (w_proj.shape[1] // n_head)
    for c in range(n_cores):
        g = c % groups
        in_maps[c]["wp"] = np.ascontiguousarray(w_proj[g * C:(g + 1) * C, :])
    return in_maps


def gather_outputs(results, b_proj, B, S, D, n_cores=N_CORES, n_head=N_HEAD):
    groups = n_cores // B
    Hg = n_head // groups
    dh = D // n_head
    out = np.zeros((B, S, D), dtype=np.float32)
    present = np.zeros((2, B, n_head, S, dh), dtype=np.float32)
    for c in range(n_cores):
        b, g = divmod(c, groups)
        out[b] += results[c]["out_p"]
        present[0, b, g * Hg:(g + 1) * Hg] = results[c]["pk"]
        present[1, b, g * Hg:(g + 1) * Hg] = results[c]["pv"]
    out += np.asarray(b_proj, np.float32)[None, None, :]
    return out, present


_NC_CACHE = {}


def _get_nc():
    if "nc" not in _NC_CACHE:
        _NC_CACHE["nc"] = build_core_program(
            S=S_, D=D_, H_loc=N_HEAD * B_ // N_CORES, dh=D_ // N_HEAD,
            n_cores=N_CORES)
    return _NC_CACHE["nc"]


def kernel(x, w_attn, b_attn, w_proj, b_proj):
    from concourse import bass_utils
    x = np.asarray(x, np.float32)
    w_attn = np.asarray(w_attn, np.float32)
    b_attn = np.asarray(b_attn, np.float32)
    w_proj = np.asarray(w_proj, np.float32)
    b_proj = np.asarray(b_proj, np.float32)
    B, S, D = x.shape

    nc = _get_nc()
    in_maps = shard_inputs(x, w_attn, b_attn)
    in_maps = shard_wproj(w_proj, in_maps)
    res = bass_utils.run_bass_kernel_spmd(nc, in_maps, core_ids=list(range(N_CORES)))
    return gather_outputs(res.results, b_proj, B, S, D)


# revision 37
# speedup vs baseline: 1.0557x; 1.0162x over previous
"""Sharded causal self-attention block (GPT-2 style) for 8 Trainium2 NeuronCores.

kernel(x, w_attn, b_attn, w_proj, b_proj) -> (out, present)
  x       [4, 2048, 1024] f32
  w_attn  [1024, 3072] f32 (fan_in_fan_out: y = x @ W + b), b_attn [3072]
  w_proj  [1024, 1024] f32, b_proj [1024]
  out     [4, 2048, 1024] f32
  present [2, 4, 16, 2048, 64] f32  (k then v, [B,H,S,dh])

Sharding: core c -> batch c//2, head-group c%2 (8 of 16 heads each).  Each core
computes qkv for its heads, causal attention, and a partial output projection
over its 512 channels; the host sums the two per-batch partials and adds b_proj.

Per-core device program (all matmuls in float32r: full PE rate, ~1e-4 rel err):
  - x^T via PE transposes; qT/kT computed head-dim-on-partitions, v token-major
    with an appended ones column per head.
  - scores computed transposed (s[j,i] = k_j.q_i) per 128x512 causal block, two
    heads paired in one 2-bank PSUM tile; exp(s/8) fused from PSUM on the
    scalar engine (one instruction per pair); causal boundary masked with one
    gpsimd affine_select per head.
  - A.V matmul uses lhsT=[v | 1] (M=65): PSUM row 64 accumulates the softmax
    denominator for free; reciprocal is partition-broadcast and fused into the
    PSUM eviction (division deferred past the exp/AV matmuls).
  - attention runs i-block-outer so the output projection for an i-block's
    tokens (contraction over all heads) interleaves into the same phase.
  - AV is software-pipelined (LAG groups) so the PE never waits on the exp.
"""
import sys
if '/opt/trn_rl_repo' not in sys.path:
    sys.path.insert(0, '/opt/trn_rl_repo')

import numpy as np
from contextlib import ExitStack

import concourse.mybir as mybir
import concourse.tile as tile
from concourse import bacc
from concourse.masks import make_identity

F32 = mybir.dt.float32
AF = mybir.ActivationFunctionType
ALU = mybir.AluOpType

N_HEAD = 16
B_, S_, D_ = 4, 2048, 1024
N_CORES = 8


def build_core_program(S=2048, D=1024, H_loc=8, dh=64, n_cores=8,
                       mm_dt=mybir.dt.float32r, LAG=2, CREDIT=0.5,
                       PAV_BUFS=2, PPJ_BUFS=2, EXPP_BUFS=4, IB_DESC=False, CREDIT_LATE=None):
    P = 128
    IB = 512                      # i-block (query block) size
    assert S % IB == 0 and D % P == 0 and dh == 64 and H_loc % 2 == 0
    HP = H_loc // 2               # head pairs
    CQK = H_loc * dh              # q (=k) columns per core
    CV = H_loc * dh
    CH = H_loc * dh               # proj input channels per core
    KO = D // P
    NT = S // P
    NTB = S // IB
    NIB = S // IB
    NCT = CQK // P                # col-tiles for each of q and k (== HP)
    CHT = CH // P
    NJT = IB // P
    NOH = max(1, D // IB)
    OCW = min(D, IB)

    FR = mm_dt
    nc = bacc.Bacc("TRN2", target_bir_lowering=False, debug=False,
                   num_devices=n_cores)
    x = nc.dram_tensor("x", [S, D], FR, kind="ExternalInput").ap()
    wqk = nc.dram_tensor("wqk", [D, 2 * CQK], FR, kind="ExternalInput").ap()
    wv = nc.dram_tensor("wv", [D, CV], FR, kind="ExternalInput").ap()
    wp = nc.dram_tensor("wp", [CH, D], FR, kind="ExternalInput").ap()
    bqk = nc.dram_tensor("bqk", [1, 2 * CQK], F32, kind="ExternalInput").ap()
    bv = nc.dram_tensor("bv", [1, CV], F32, kind="ExternalInput").ap()
    out_p = nc.dram_tensor("out_p", [S, D], F32, kind="ExternalOutput").ap()
    pk = nc.dram_tensor("pk", [H_loc, S, dh], F32, kind="ExternalOutput").ap()
    pv = nc.dram_tensor("pv", [H_loc, S, dh], F32, kind="ExternalOutput").ap()

    # head-major views of the present outputs: [p, ttile, head, dh]
    pk_t = pk.rearrange("h (t p) d -> p t h d", p=P)
    pv_t = pv.rearrange("h (t p) d -> p t h d", p=P)
    wqk_t = wqk.rearrange("(ko p) c -> p ko c", p=P)

    scale = float(1.0 / np.sqrt(dh))

    with tile.TileContext(nc) as tc, ExitStack() as ctx:
        st0x = ctx.enter_context(tc.tile_pool(name="st0x", bufs=3))
        # first x chunk DMAs go out before anything else
        x_chunks = {}

        def xc_prefetch(gt, NT_=S // P):
            if gt >= NT_ or gt in x_chunks:
                return
            t_ = st0x.tile([P, D], FR, tag="x_in", name=f"xc{gt}")
            nc.sync.dma_start(
                t_, x[gt * P:(gt + 1) * P].rearrange("(o p) d -> p (o d)", p=P))
            x_chunks[gt] = t_

        t0_ = st0x.tile([P, D], FR, tag="x_in", name="xc0")
        nc.sync.dma_start(t0_[:, 0:2 * P],
                          x[0:P, 0:2 * P].rearrange("(o p) d -> p (o d)", p=P))
        nc.sync.dma_start(t0_[:, 2 * P:],
                          x[0:P, 2 * P:].rearrange("(o p) d -> p (o d)", p=P))
        x_chunks[0] = t0_
        for _g in range(1, 3):
            xc_prefetch(_g)

        const = ctx.enter_context(tc.tile_pool(name="const", bufs=1))
        identity32 = const.tile([P, P], F32)
        make_identity(nc, identity32)
        identity = const.tile([P, P], FR)
        nc.vector.tensor_copy(identity, identity32)
        bqk_sb = const.tile([P, 2 * NCT], F32)
        nc.sync.dma_start(bqk_sb, bqk.rearrange("o (ct p) -> p (o ct)", p=P))
        bv_row = const.tile([1, CV], F32)
        nc.sync.dma_start(bv_row, bv)
        bv_rep = const.tile([P, CV], F32)
        nc.gpsimd.partition_broadcast(bv_rep, bv_row)

        big = ctx.enter_context(tc.tile_pool(name="big", bufs=1))
        qTp = tc.tile_pool(name="qTp", bufs=1)
        qT = qTp.__enter__().tile([P, NCT, S], FR)
        kT = big.tile([P, NCT, S], FR)
        vext = big.tile([P, NT, H_loc, dh + 1], FR)    # token-major v | ones col
        nc.vector.tensor_scalar(vext[:, :, :, dh:dh + 1],
                                bv_rep[:, 0:NT * H_loc], 0.0, 1.0,
                                ALU.mult, ALU.add)

        # ---------------- stage 0: x^T, qkv (two 512-blocks per wqk load) ---
        TP = 2 * IB               # token-pair block (1024)
        NTP = S // TP
        with tc.tile_pool(name="st0t", bufs=1) as st0t, \
             tc.tile_pool(name="wvp", bufs=2) as wvp, \
             tc.tile_pool(name="wqp", bufs=3) as wqp, \
             tc.tile_pool(name="ps0", bufs=2, space="PSUM") as ps0, \
             tc.tile_pool(name="pstr", bufs=4, space="PSUM") as pstr:
            for T in range(NTP):
                xT = st0t.tile([P, KO, TP], FR, tag="xT")
                wts = {}

                def wt_prefetch(ct):
                    wt = wqp.tile([P, KO, P], FR, tag="wqk", name=f"wt{ct}")
                    nc.sync.dma_start(wt, wqk_t[:, :, ct * P:(ct + 1) * P])
                    wts[ct] = wt

                wt_prefetch(0)
                for tcn in range(TP // P):
                    gt = T * (TP // P) + tcn
                    xc_prefetch(gt)
                    xc = x_chunks.pop(gt)
                    if tcn == TP // P - 2:
                        wt_prefetch(1)
                    else:
                        xc_prefetch(gt + 2)
                    for dt_ in range(KO):
                        pst = pstr.tile([P, P], FR, tag="tr")
                        nc.tensor.transpose(pst, xc[:, dt_ * P:(dt_ + 1) * P],
                                            identity)
                        if dt_ % 2 == 0:
                            nc.scalar.copy(xT[:, dt_, tcn * P:(tcn + 1) * P], pst)
                        else:
                            nc.vector.tensor_copy(
                                xT[:, dt_, tcn * P:(tcn + 1) * P], pst)
                # q and k column tiles (transposed orientation)
                for ct in range(2 * NCT):
                    if ct + 2 < 2 * NCT:
                        wt_prefetch(ct + 2)
                    # pull next T-pair's x chunks in during the matmul phase
                    xc_prefetch((T + 1) * (TP // P) + ct)
                    wt = wts.pop(ct)
                    for half in range(TP // IB):
                        ps = ps0.tile([P, IB], F32, tag="mm")
                        for k in range(KO):
                            nc.tensor.matmul(
                                ps, wt[:, k],
                                xT[:, k, half * IB:(half + 1) * IB],
                                start=(k == 0), stop=(k == KO - 1))
                        dest = (qT if ct < NCT else kT)[
                            :, ct % NCT, T * TP + half * IB:T * TP + (half + 1) * IB]
                        nc.vector.tensor_scalar_add(dest, ps, bqk_sb[:, ct:ct + 1])
                # v (token-major, wv streamed in halves) + present-v out
                for vh in range(2):
                    wvt = wvp.tile([P, KO, CV // 2], FR, tag="wv")
                    nc.sync.dma_start(
                        wvt, wv.rearrange("(ko p) c -> p ko c", p=P)
                        [:, :, vh * (CV // 2):(vh + 1) * (CV // 2)])
                    for tcn in range(TP // P):
                        ps = ps0.tile([P, CV // 2], F32, tag="mmv", bufs=2)
                        for k in range(KO):
                            nc.tensor.matmul(ps, xT[:, k, tcn * P:(tcn + 1) * P],
                                             wvt[:, k],
                                             start=(k == 0), stop=(k == KO - 1))
                        gt = T * (TP // P) + tcn
                        h0 = vh * (H_loc // 2)
                        nc.vector.tensor_tensor(
                            vext[:, gt, h0:h0 + H_loc // 2, 0:dh], ps,
                            bv_rep[:, vh * (CV // 2):(vh + 1) * (CV // 2)], ALU.add)

        # ---------------- stage A: attention + proj (i-block outer) --------
        aT = big.tile([P, CHT, S], FR)
        with tc.tile_pool(name="expp", bufs=EXPP_BUFS) as expp, \
             tc.tile_pool(name="dpool", bufs=2) as dpool, \
             tc.tile_pool(name="rrpool", bufs=2) as rrpool, \
             tc.tile_pool(name="kout", bufs=3) as kout, \
             tc.tile_pool(name="wpp", bufs=1) as wpp, \
             tc.tile_pool(name="outp", bufs=3) as outpool, \
             tc.tile_pool(name="psc", bufs=2, space="PSUM") as psc, \
             tc.tile_pool(name="pav", bufs=PAV_BUFS, space="PSUM") as pav, \
             tc.tile_pool(name="ppj", bufs=PPJ_BUFS, space="PSUM") as ppj:
            wp_sb = wpp.tile([P, CHT, D], FR)
            nc.sync.dma_start(wp_sb, wp.rearrange("(ko p) c -> p ko c", p=P))

            def pk_unit(hp, jt):
                pst = ppj.tile([P, P], FR, tag="fill", name="pkt")
                nc.tensor.transpose(pst, kT[:, hp, jt * P:(jt + 1) * P],
                                    identity)
                ko = kout.tile([P, P], F32, tag="ko")
                nc.any.tensor_copy(ko, pst[:, 0:P])
                nc.sync.dma_start(pk_t[:, jt, 2 * hp:2 * hp + 2, :], ko)

            def proj_unit(tt, oh):
                ps = ppj.tile([P, OCW], F32, tag="fill", name="pjt")
                for k in range(CHT):
                    nc.tensor.matmul(ps[:, 0:OCW], aT[:, k, tt * P:(tt + 1) * P],
                                     wp_sb[:, k, oh * OCW:(oh + 1) * OCW],
                                     start=(k == 0), stop=(k == CHT - 1))
                ot = outpool.tile([P, OCW], F32, tag="ot")
                nc.any.tensor_copy(ot, ps[:, 0:OCW])
                nc.sync.dma_start(
                    out_p[tt * P:(tt + 1) * P, oh * OCW:(oh + 1) * OCW], ot)

            # filler queue: proj of completed i-blocks + present-k transposes
            # + present-v DMAs, sprinkled between attention groups to absorb
            # PE wait slots
            fillers = []
            fill_state = {"credit": 0.0}
            for hp_ in range(HP):
                for jt_ in range(NT):
                    fillers.append(lambda hp2=hp_, jt=jt_: pk_unit(hp2, jt))
                    if jt_ % 4 == 0:
                        gt = hp_ * 4 + jt_ // 4
                        fillers.append(lambda gt=gt: nc.sync.dma_start(
                            pv_t[:, gt], vext[:, gt, :, 0:dh].bitcast(F32)))

            def emit_fillers(credit):
                fill_state["credit"] += credit
                while fillers and fill_state["credit"] >= 1.0:
                    fillers.pop(0)()
                    fill_state["credit"] -= 1.0

            prev_drain = [None]
            for ib in (range(NIB - 1, -1, -1) if IB_DESC else range(NIB)):
                njt = NJT * ib + NJT
                for hp in range(HP):
                    av = [pav.tile([P, IB], F32, tag="av", name=f"av{_X}")
                          for _X in range(2)]

                    def emit_av(j, ex, cs, njt=njt, hp=hp, av=av):
                        for X in range(2):
                            nc.tensor.matmul(
                                av[X][0:dh + 1, cs],
                                vext[:, j, 2 * hp + X, :],
                                ex[:, X, cs],
                                start=(j == 0), stop=(j == njt - 1))

                    pending = []
                    for j in range(njt):
                        delta = j * P - ib * IB
                        # pad restricted blocks to >=256 cols (fp32r rate rule)
                        lo = min(max(0, delta), IB - 256)
                        cs = slice(lo, IB)
                        ecs = slice(max(0, delta), IB)
                        ssc = psc.tile([P, 2, IB], F32, tag="sc")
                        for X in range(2):
                            b0 = X * 64
                            nc.tensor.matmul(
                                ssc[:, X, cs],
                                kT[b0:b0 + 64, hp, j * P:(j + 1) * P],
                                qT[b0:b0 + 64, hp, ib * IB:(ib + 1) * IB][:, cs],
                                start=True, stop=True)
                        ex = expp.tile([P, 2, IB], FR, tag="exp")
                        nc.scalar.activation(ex[:, :, ecs], ssc[:, :, ecs], AF.Exp,
                                             scale=scale)
                        if delta >= 0:
                            # zero cols [lo, delta) and the triangular part of
                            # [delta, delta+P): keep iff (c - delta - p) >= 0
                            for X in range(2):
                                nc.gpsimd.affine_select(
                                    out=ex[:, X, lo:delta + P],
                                    in_=ex[:, X, lo:delta + P],
                                    compare_op=ALU.is_ge, fill=0.0,
                                    base=lo - delta, channel_multiplier=-1,
                                    pattern=[[1, delta + P - lo]])
                        pending.append((j, ex, cs))
                        # after this unit's first score-group is in flight,
                        # drain the previous unit (keeps ACT fed at the switch)
                        if j == 0 and prev_drain[0] is not None:
                            prev_drain[0]()
                            prev_drain[0] = None
                        if len(pending) > LAG:
                            emit_av(*pending.pop(0))
                        cr = CREDIT if (CREDIT_LATE is None or ib < 2) \
                            else CREDIT_LATE
                        emit_fillers(cr)

                    def drain(pending=pending, av=av, emit_av=emit_av,
                              hp=hp, ib=ib):
                        for p_ in pending:
                            emit_av(*p_)
                        for X in range(2):
                            avs = dpool.tile([dh, IB], F32, tag="avs")
                            nc.vector.tensor_copy(avs, av[X][0:dh, :])
                            dX = dpool.tile([1, IB], F32, tag="d")
                            nc.vector.tensor_copy(dX, av[X][dh:dh + 1, :])
                            nc.vector.reciprocal(dX, dX)
                            rr = rrpool.tile([64, IB], F32, tag="rr")
                            nc.gpsimd.partition_broadcast(rr, dX)
                            nc.vector.tensor_tensor(
                                aT[X * 64:X * 64 + 64, hp, ib * IB:(ib + 1) * IB],
                                avs[0:dh, :], rr, ALU.mult)
                        emit_fillers(1.0)
                    prev_drain[0] = drain
                # queue fillers: this i-block's proj + one head-pair present-k
                for tt in range(ib * NJT, (ib + 1) * NJT):
                    for oh in range(NOH):
                        fillers.append(lambda tt=tt, oh=oh: proj_unit(tt, oh))
            if prev_drain[0] is not None:
                prev_drain[0]()
                prev_drain[0] = None
            for f in fillers:
                f()
        qTp.__exit__(None, None, None)

    nc.compile()
    return nc


def shard_inputs(x, w_attn, b_attn, n_cores=N_CORES, n_head=N_HEAD):
    """Hybrid shard: core c -> batch c//2, head-group c%2."""
    B, S, D = x.shape
    dh = D // n_head
    groups = n_cores // B
    Hg = n_head // groups
    C = Hg * dh
    in_maps = []
    for c in range(n_cores):
        b, g = divmod(c, groups)
        base = g * C
        wqk_c = np.ascontiguousarray(
            np.concatenate([w_attn[:, base:base + C],
                            w_attn[:, D + base:D + base + C]], axis=1))
        wv_c = np.ascontiguousarray(w_attn[:, 2 * D + base:2 * D + base + C])
        bqk_c = np.ascontiguousarray(
            np.concatenate([b_attn[base:base + C],
                            b_attn[D + base:D + base + C]])[None, :])
        bv_c = np.ascontiguousarray(b_attn[2 * D + base:2 * D + base + C][None, :])
        in_maps.append({"x": np.ascontiguousarray(x[b]), "wqk": wqk_c,
                        "wv": wv_c, "bqk": bqk_c, "bv": bv_c})
    return in_maps


def shard_wproj(w_proj, in_maps, n_cores=N_CORES, n_head=N_HEAD, B=B_):
    groups = n_cores // B
    C = (n_head // groups) * (w_proj.shape[1] // n_head)
    for c in range(n_cores):
        g = c % groups
        in_maps[c]["wp"] = np.ascontiguousarray(w_proj[g * C:(g + 1) * C, :])
    return in_maps


def gather_outputs(results, b_proj, B, S, D, n_cores=N_CORES, n_head=N_HEAD):
    groups = n_cores // B
    Hg = n_head // groups
    dh = D // n_head
    out = np.zeros((B, S, D), dtype=np.float32)
    present = np.zeros((2, B, n_head, S, dh), dtype=np.float32)
    for c in range(n_cores):
        b, g = divmod(c, groups)
        out[b] += results[c]["out_p"]
        present[0, b, g * Hg:(g + 1) * Hg] = results[c]["pk"]
        present[1, b, g * Hg:(g + 1) * Hg] = results[c]["pv"]
    out += np.asarray(b_proj, np.float32)[None, None, :]
    return out, present


_NC_CACHE = {}


def _get_nc():
    if "nc" not in _NC_CACHE:
        _NC_CACHE["nc"] = build_core_program(
            S=S_, D=D_, H_loc=N_HEAD * B_ // N_CORES, dh=D_ // N_HEAD,
            n_cores=N_CORES)
    return _NC_CACHE["nc"]


def kernel(x, w_attn, b_attn, w_proj, b_proj):
    from concourse import bass_utils
    x = np.asarray(x, np.float32)
    w_attn = np.asarray(w_attn, np.float32)
    b_attn = np.asarray(b_attn, np.float32)
    w_proj = np.asarray(w_proj, np.float32)
    b_proj = np.asarray(b_proj, np.float32)
    B, S, D = x.shape

    nc = _get_nc()
    in_maps = shard_inputs(x, w_attn, b_attn)
    in_maps = shard_wproj(w_proj, in_maps)
    res = bass_utils.run_bass_kernel_spmd(nc, in_maps, core_ids=list(range(N_CORES)))
    return gather_outputs(res.results, b_proj, B, S, D)
